# revision 74
# baseline (speedup 1.0000x reference)
"""Bass/Trainium2 kernel for nn_ButterflyGatingUnit (v2).

Data-parallel over batch B=8 across 8 NeuronCores (one image per core).

v2 redesign vs v1 (307us -> target ~230us):
  - im2col is built on the HOST into two DRAM tensors (one per input half),
    with the per-fill pixel ordering PHASE-GROUPED: free index within a
    12-row fill is (ty, tx, j, bx) where the output pixel is
    (12r + 3j + ty, 3bx + tx).  One big DMA per (input, fill) replaces
    ~26 small DMAs, and every 128-slice of the free dim is exactly one
    3x3 phase class = one attention tap's block positions for that fill.
  - q and k convs run in TRANSPOSED orientation: lhsT = im2col tile
    (m = 128 pixels, full PE rows), rhs = [wq|wk] weights (free = 192).
    The psum windows come out as qT/kT chunks [128 positions, 192] that
    feed the gram matmuls directly -- no DMA transposes at all.
  - Per-tap gram matrices accumulate in two persistent PSUM banks across
    all 8 fills (one start=True per bank; the bank's pending-zero covers
    the other taps' first write), so the attention setup leaves the
    critical path.
  - y1/y2 stay in SBUF as bf16 (phase-ordered; no DRAM round trip).  The
    epilogue is all-bf16 (x is staged as bf16, out is written bf16 and
    widened on the host), computed in SBUF with the phase->raster
    un-permute folded into the final add's access patterns, engine-
    balanced so it hides under attn@v.
  - v is un-permuted to a raster padded buffer at evacuation (3 strided
    copies per psum tile on Pool); attn @ v is the same dynamic-weight
    3x3 tap conv as v1.

All heavy matmuls are bf16 with fp32 PSUM accumulation; stats, softmax,
and BN coefficients are fp32.
"""

import os
import sys

sys.path.insert(0, "/opt/trn_rl_repo")

import numpy as np

import concourse.bass as bass
import concourse.mybir as mybir
import concourse.tile as tile
from concourse.masks import make_identity

F32 = mybir.dt.float32
BF16 = mybir.dt.bfloat16
AF = mybir.ActivationFunctionType
ALU = mybir.AluOpType
AX = mybir.AxisListType

B, C, H, W = 8, 96, 96, 96
HW = H * W                 # 9216
HP = H + 2                 # 98
NPIX = float(B * HW)       # BN count over (B,H,W)
EPS = 1e-5
S_ATTN = float(np.sqrt(C * 9.0))   # softmax scale sqrt(864)
NF = 8                     # fills (12 rows each)
FILL = HW // NF            # 1152 pixels per fill
NROW4 = 4                  # rows per attn psum tile (N = 384)
NG4 = H // NROW4           # 24 attn@v groups

# ---------------------------------------------------------------------------
# Workaround for this walrus build: only ONE sem wait is encodable per
# instruction. After Tile assigns waits, move extras onto fresh same-engine
# NoOps inserted right before the instruction (same engine + program order
# => identical blocking semantics).
_MAXW = 1


def _split_multiwaits(nc):
    for f in nc.m.functions:
        for bb in f.blocks:
            insts = bb.instructions
            if not any(
                i.sync_info is not None and len(i.sync_info.on_wait) > _MAXW
                for i in insts
            ):
                continue
            new = []
            for inst in insts:
                si = inst.sync_info
                if si is not None and len(si.on_wait) > _MAXW:
                    waits = list(si.on_wait)
                    keep, rest = waits[:_MAXW], waits[_MAXW:]
                    while rest:
                        nop = mybir.InstNoOp(name=f"I-waitsplit-{nc.next_id()}")
                        nop.engine = inst.engine
                        nop.sync_info = mybir.SyncInfo(
                            on_wait=rest[:_MAXW], on_update=[]
                        )
                        rest = rest[_MAXW:]
                        new.append(nop)
                    inst.sync_info = mybir.SyncInfo(
                        on_wait=keep, on_update=list(si.on_update)
                    )
                new.append(inst)
            bb.instructions = new


_orig_drain_and_barrier = tile.TileContext._drain_and_barrier


def _patched_drain_and_barrier(self, tick_clock, wait_clock):
    _orig_drain_and_barrier(self, tick_clock, wait_clock)
    _split_multiwaits(self.nc)


tile.TileContext._drain_and_barrier = _patched_drain_and_barrier
# ---------------------------------------------------------------------------


def build_nc_v2(collectives=True):
    nc = bass.Bass(num_devices=8)

    im1_d = nc.dram_tensor("im1", [128, 7, HW], BF16, kind="ExternalInput")
    im2_d = nc.dram_tensor("im2", [128, 7, HW], BF16, kind="ExternalInput")
    x1f_d = nc.dram_tensor("x1f", [C, HW], BF16, kind="ExternalInput")
    x2f_d = nc.dram_tensor("x2f", [C, HW], BF16, kind="ExternalInput")
    w1i_d = nc.dram_tensor("w1i", [128, 7, C], BF16, kind="ExternalInput")
    w2i_d = nc.dram_tensor("w2i", [128, 7, C], BF16, kind="ExternalInput")
    wqk_d = nc.dram_tensor("wqk", [128, 2, 7, 3 * C], BF16, kind="ExternalInput")
    out_d = nc.dram_tensor("out", [2 * C, HW], BF16, kind="ExternalOutput")

    with tile.TileContext(nc) as tc:
        with (
            tc.tile_pool(name="cst", bufs=1) as cst,
            tc.tile_pool(name="im", bufs=4) as im,
            tc.tile_pool(name="reuse", bufs=4) as reuse,
            tc.tile_pool(name="scr", bufs=1) as scr,
            tc.tile_pool(name="ps", bufs=4, space="PSUM") as ps,
            tc.tile_pool(name="psq", bufs=2, space="PSUM") as psq,
            tc.tile_pool(name="gram", bufs=1, space="PSUM") as gram,
            tc.tile_pool(name="dram", bufs=1, space="DRAM") as dram,
        ):
            w1i = cst.tile([128, 7, C], BF16)
            wqk = cst.tile([128, 2, 7, 3 * C], BF16)
            w2i = cst.tile([128, 7, C], BF16)

            # phase-ordered bf16 conv outputs kept in SBUF
            y1b = cst.tile([C, HW], BF16)
            y2b = cst.tile([C, HW], BF16)
            # qT/kT chunks: [pos-in-chunk, tap, fill, 0:96=q | 96:192=k]
            qkT = cst.tile([128, 9, NF, 2 * C], BF16)
            # raster padded v for the dynamic tap conv
            vpad = cst.tile([C, HP, HP], BF16)
            nc.gpsimd.memset(vpad[:, 0, :], 0.0)
            nc.gpsimd.memset(vpad[:, HP - 1, :], 0.0)
            nc.gpsimd.memset(vpad[:, 1 : HP - 1, 0], 0.0)
            nc.gpsimd.memset(vpad[:, 1 : HP - 1, HP - 1], 0.0)

            identb128 = cst.tile([128, 128], BF16)
            make_identity(nc, identb128)
            eps_t = cst.tile([C, 1], F32)
            nc.gpsimd.memset(eps_t[:], EPS)
            y3b = cst.tile([C, HW], BF16)
            st = cst.tile([C, 3, NG4], F32)
            stq = cst.tile([C, 3, NG4], F32)

            # persistent gram accumulators: taps 0-4 and 5-8
            gram1 = gram.tile([C, 5 * C], F32)
            gram2 = gram.tile([C, 4 * C], F32)

            def mm_norm(pt, lhs_of, IMs, fsl, halves):
                i_last = len(halves) * 7 - 1
                i = 0
                for hi in halves:
                    for k in range(7):
                        Kk = 128 if k < 6 else 96
                        nc.tensor.matmul(
                            pt[:],
                            lhs_of(hi, k)[0:Kk, :],
                            IMs[hi][0:Kk, k, fsl],
                            start=(i == 0),
                            stop=(i == i_last),
                        )
                        i += 1

            def emit_gram(rp, taps=range(9)):
                # one matmul per tap accumulating chunk rp into the
                # persistent gram banks; exactly one start per BANK (the
                # pending-zero region covers the other taps' first write).
                for t in taps:
                    if t < 5:
                        out_ap = gram1[:, t * C : (t + 1) * C]
                        first = rp == 0 and t == 0
                        last = rp == NF - 1 and t == 4
                    else:
                        out_ap = gram2[:, (t - 5) * C : (t - 4) * C]
                        first = rp == 0 and t == 5
                        last = rp == NF - 1 and t == 8
                    nc.tensor.matmul(
                        out_ap,
                        qkT[:, t, rp, C : 2 * C],
                        qkT[:, t, rp, 0:C],
                        start=first,
                        stop=last,
                        skip_group_check=True,
                    )

            def qk_windows(r, i, IMs, vsb):
                for t in range(3 * i, 3 * i + 3):
                    pqf = psq.tile([128, 3 * C], F32, tag="qk", name="qk")
                    pq = pqf[:, :]
                    mi = 0
                    for hi, IMh in enumerate(IMs):
                        for k in range(7):
                            Kk = 128 if k < 6 else 96
                            nc.tensor.matmul(
                                pq,
                                IMh[0:Kk, k, t * 128 : (t + 1) * 128],
                                wqk[0:Kk, hi, k, :],
                                start=(mi == 0),
                                stop=(mi == 13),
                            )
                            mi += 1
                    if t % 2 == 0:
                        nc.vector.tensor_copy(qkT[:, t, r, :], pq[:, 0 : 2 * C])
                        nc.scalar.activation(
                            out=vsb[:, t, :], in_=pq[:, 2 * C : 3 * C],
                            func=AF.Copy,
                        )
                    else:
                        nc.scalar.activation(
                            out=qkT[:, t, r, :], in_=pq[:, 0 : 2 * C],
                            func=AF.Copy,
                        )
                        nc.vector.tensor_copy(vsb[:, t, :], pq[:, 2 * C : 3 * C])

            def v_transposes(r, i, vsb):
                # vT windows -> raster vpad via PE transpose per tap
                vwin = vpad[:, 1 + 12 * r : 13 + 12 * r, 1 : 1 + W].rearrange(
                    "p (j ty) (bx tx) -> p ty tx j bx", j=4, ty=3, bx=32, tx=3
                )
                for t in range(3 * i, 3 * i + 3):
                    ty, tx = t // 3, t % 3
                    pvf = ps.tile([C, 384], BF16, tag="convps", name="convps")
                    pv = pvf[:, 0:128]
                    nc.tensor.transpose(pv, vsb[:, t, :], identb128[:])
                    src = pv.rearrange("p (j bx) -> p j bx", j=4)
                    if t % 2 == 0:
                        nc.vector.tensor_copy(vwin[:, ty, tx], src)
                    else:
                        nc.scalar.activation(
                            out=vwin[:, ty, tx], in_=src, func=AF.Copy
                        )

            def conv_third(r, i, IMs, vsb, qk_first=False):
                """One 384-pixel third of a fill."""
                fsl = slice(i * 384, (i + 1) * 384)
                g = 3 * r + i
                if qk_first:
                    qk_windows(r, i, IMs, vsb)
                    emit_gram(r, range(3 * i, 3 * i + 3))
                # ---- y1 (x1 only), phase-ordered evac + stats ----
                ptf = ps.tile([C, 384], F32, tag="convps", name="convps")
                pt = ptf[:, :]
                mm_norm(pt, lambda h, k: w1i[:, k, :], IMs, fsl, [0])
                if qk_first:
                    # keep fill 7's ACT queue clear so the softmax exps fire
                    # as soon as the grams close
                    nc.vector.tensor_scalar(
                        out=y1b[:, r * FILL + i * 384 : r * FILL + (i + 1) * 384],
                        in0=pt[:], scalar1=1.0, scalar2=0.0,
                        op0=ALU.mult, op1=ALU.add,
                        accum_out=st[:, 0, g : g + 1],
                    )
                else:
                    nc.scalar.activation(
                        out=y1b[:, r * FILL + i * 384 : r * FILL + (i + 1) * 384],
                        in_=pt[:], func=AF.Copy, accum_out=st[:, 0, g : g + 1],
                    )
                y1sl = y1b[:, r * FILL + i * 384 : r * FILL + (i + 1) * 384]
                sq = scr.tile([C, 384], F32, tag="sqscr", name="sq")
                if qk_first:
                    sqb = scr.tile([C, 384], BF16, tag="sqb", name="sqb")
                    nc.vector.scalar_tensor_tensor(
                        out=sqb[:], in0=y1sl, scalar=1.0, in1=y1sl,
                        op0=ALU.mult, op1=ALU.mult,
                        accum_out=stq[:, 0, g : g + 1],
                    )
                else:
                    nc.scalar.activation(
                        out=sq[:], in_=pt[:], func=AF.Square,
                        accum_out=stq[:, 0, g : g + 1],
                    )
                if not qk_first:
                    qk_windows(r, i, IMs, vsb)
                if i > 0:
                    v_transposes(r, i - 1, vsb)
                # ---- y2 (x2 only) ----
                ptf = ps.tile([C, 384], F32, tag="convps", name="convps")
                pt = ptf[:, :]
                mm_norm(pt, lambda h, k: w2i[:, k, :], IMs, fsl, [1])
                nc.vector.tensor_scalar(
                    out=y2b[:, r * FILL + i * 384 : r * FILL + (i + 1) * 384],
                    in0=pt[:], scalar1=1.0, scalar2=0.0,
                    op0=ALU.mult, op1=ALU.add,
                    accum_out=st[:, 1, g : g + 1],
                )
                y2sl = y2b[:, r * FILL + i * 384 : r * FILL + (i + 1) * 384]
                sq = scr.tile([C, 384], BF16, tag="sqscr", name="sq")
                nc.vector.scalar_tensor_tensor(
                    out=sq[:], in0=y2sl, scalar=1.0, in1=y2sl,
                    op0=ALU.mult, op1=ALU.mult,
                    accum_out=stq[:, 1, g : g + 1],
                )

            for r in range(NF):
                IM1 = im.tile([128, 7, FILL], BF16, tag="im", name="im1")
                IM2 = im.tile([128, 7, FILL], BF16, tag="im", name="im2")
                sl_r = slice(r * FILL, (r + 1) * FILL)
                if r == 0:
                    # interleave piecewise so compute starts after ~2us;
                    # later-needed weights load between the pieces
                    nc.sync.dma_start(IM1[:, :, 0:384], im1_d[:, :, 0:384])
                    nc.scalar.dma_start(w1i[:], w1i_d[:])
                    nc.sync.dma_start(IM2[:, :, 0:384], im2_d[:, :, 0:384])
                    nc.scalar.dma_start(wqk[:], wqk_d[:])
                    nc.sync.dma_start(IM1[:, :, 384:768], im1_d[:, :, 384:768])
                    nc.sync.dma_start(IM2[:, :, 384:768], im2_d[:, :, 384:768])
                    nc.scalar.dma_start(w2i[:], w2i_d[:])
                    nc.sync.dma_start(IM1[:, :, 768:FILL], im1_d[:, :, 768:FILL])
                    nc.sync.dma_start(IM2[:, :, 768:FILL], im2_d[:, :, 768:FILL])
                else:
                    nc.sync.dma_start(IM1[:], im1_d[:, :, sl_r])
                    nc.sync.dma_start(IM2[:], im2_d[:, :, sl_r])
                if r == 1:
                    # prefill out halves with x; epilogue accum-DMAs add the
                    # bn terms on top
                    nc.scalar.dma_start(out_d[0:C, :], x1f_d[:])

                # gram partials for the previous fill's chunk; the last
                # fill's own partials are interleaved with its qk windows,
                # so its predecessor must be accumulated first
                vsb = im.tile([128, 9, C], BF16, tag="vsb", bufs=2,
                              name="vsb")
                for i in range(3):
                    conv_third(r, i, [IM1, IM2], vsb, qk_first=False)
                if r == NF - 1:
                    # let the gram close before the last v transposes so
                    # the softmax exps fire as early as possible
                    emit_gram(r - 1)
                    emit_gram(r)
                    v_transposes(r, 2, vsb)
                else:
                    v_transposes(r, 2, vsb)
                    if r > 0:
                        emit_gram(r - 1)

            # ------- softmax: exp of grams, denominator, transposes -------
            # No max shift: logits/sqrt(864) are O(few), safe for fp32 exp.
            A = cst.tile([C, 9, C], BF16)
            attnT = cst.tile([C, 9, C], BF16)
            identf = cst.tile([C, C], BF16)
            make_identity(nc, identf)
            def transpose_tap(t):
                ppf = ps.tile([C, 384], BF16, tag="convps", name="convps")
                pp = ppf[:, 0:C]
                nc.tensor.transpose(pp, A[:, t, :], identf[:])
                if t % 2 == 0:
                    nc.scalar.activation(out=attnT[:, t, :], in_=pp, func=AF.Copy)
                else:
                    nc.vector.tensor_copy(attnT[:, t, :], pp)

            nc.scalar.activation(
                A[:, 0:5, :].rearrange("p a b -> p (a b)"), gram1[:, :],
                AF.Exp, scale=1.0 / S_ATTN,
            )
            for t in range(5):
                transpose_tap(t)
            nc.scalar.activation(
                A[:, 5:9, :].rearrange("p a b -> p (a b)"), gram2[:, :],
                AF.Exp, scale=1.0 / S_ATTN,
            )
            for t in range(5, 9):
                transpose_tap(t)
            Aflat = A[:].rearrange("p a b -> p (a b)")
            dsum = cst.tile([C, 1], F32)
            nc.vector.reduce_sum(dsum[:], Aflat, axis=AX.X)
            rd = cst.tile([C, 1], F32)
            nc.vector.reciprocal(rd[:], dsum[:])

            # ---------------- stats collective #1 (y1, y2) --------------
            stats1 = cst.tile([C, 4], F32)
            nc.vector.reduce_sum(stats1[:, 0:1], st[:, 0, :], axis=AX.X)
            nc.vector.reduce_sum(stats1[:, 1:2], stq[:, 0, :], axis=AX.X)
            nc.vector.reduce_sum(stats1[:, 2:3], st[:, 1, :], axis=AX.X)
            nc.vector.reduce_sum(stats1[:, 3:4], stq[:, 1, :], axis=AX.X)
            cc1_in = dram.tile([C, 4], F32)
            cc1_out = dram.tile([C, 4], F32)
            nc.sync.dma_start(cc1_in[:], stats1[:])
            if collectives:
                nc.gpsimd.collective_compute(
                    "AllReduce", ALU.add, replica_groups=[list(range(8))],
                    ins=[cc1_in[:].opt()], outs=[cc1_out[:].opt()],
                )
            else:
                nc.sync.dma_start(cc1_out[:], cc1_in[:])
            stats1r = cst.tile([C, 4], F32)
            nc.sync.dma_start(stats1r[:], cc1_out[:])

            def bn_coeffs(sum_col, sq_col, label):
                mu = cst.tile([C, 1], F32, tag=f"mu_{label}", name=f"mu_{label}")
                nc.vector.tensor_scalar_mul(mu[:], sum_col, 1.0 / NPIX)
                ex2 = cst.tile([C, 1], F32, tag=f"e2_{label}", name=f"e2_{label}")
                nc.vector.tensor_scalar_mul(ex2[:], sq_col, 1.0 / NPIX)
                # nv = mu^2 - E[y^2] = -var; sd = sqrt(-1*nv + eps)
                nv = cst.tile([C, 1], F32, tag=f"v_{label}", name=f"v_{label}")
                nc.vector.scalar_tensor_tensor(
                    out=nv[:], in0=mu[:], scalar=mu[:], in1=ex2[:],
                    op0=ALU.mult, op1=ALU.subtract,
                )
                sd = cst.tile([C, 1], F32, tag=f"s_{label}", name=f"s_{label}")
                nc.scalar.activation(sd[:], nv[:], AF.Sqrt, scale=-1.0, bias=eps_t[:])
                r_ = cst.tile([C, 1], F32, tag=f"r_{label}", name=f"r_{label}")
                nc.vector.reciprocal(r_[:], sd[:])
                mb = None
                if label in ("y2", "y3"):
                    mb = cst.tile([C, 1], F32, tag=f"m_{label}",
                                  name=f"m_{label}")
                    nc.vector.tensor_scalar(
                        out=mb[:], in0=mu[:], scalar1=r_[:], scalar2=-1.0,
                        op0=ALU.mult, op1=ALU.mult,
                    )
                return mu, r_, mb

            mu1, r1, mb1 = bn_coeffs(stats1r[:, 0:1], stats1r[:, 1:2], "y1")
            mu2, r2, mb2 = bn_coeffs(stats1r[:, 2:3], stats1r[:, 3:4], "y2")

            # ------------- attn @ v with interleaved epilogue-1 ----------
            # out[0:96] = x1 + bn(y1)*bn(y2), all bf16, computed in SBUF and
            # written once; the phase->raster un-permute rides on the final
            # add's access patterns.  Engine split per chunk: t1 ACT,
            # t2 DVE, g12 Pool, un-permute adds DVE, write on SP.
            # combined scalars: gated = (y1-mu1)*(rc*y2 + bc),
            # rc = r1*r2, bc = -mu2*rc
            rc = cst.tile([C, 1], F32)
            nc.vector.tensor_tensor(rc[:], r1[:], r2[:], ALU.mult)
            bc = cst.tile([C, 1], F32)
            nc.vector.tensor_scalar(
                out=bc[:], in0=mu2[:], scalar1=rc[:], scalar2=-1.0,
                op0=ALU.mult, op1=ALU.mult,
            )

            def epi1_chunk(c0):
                sl = slice(c0 * FILL, (c0 + 1) * FILL)
                t2 = reuse.tile([C, FILL], BF16, tag="reuse")
                nc.scalar.activation(
                    t2[:], y2b[:, sl], AF.Identity, bias=bc[:], scale=rc[:]
                )
                # g12 = (y1-mu1)*t2 with the phase->raster un-permute on the
                # dst AP (split per ty to keep APs at 4 dims)
                g12 = reuse.tile([C, FILL], BF16, tag="reuse")
                # HW limits stt outputs to <=2 free dims: iterate (ty, j),
                # each piece is [p, tx, bx]
                t1v = y1b[:, sl].rearrange("p (ty tx j bx) -> p ty j tx bx",
                                           ty=3, tx=3, j=4, bx=32)
                t2v = t2[:].rearrange("p (ty tx j bx) -> p ty j tx bx",
                                      ty=3, tx=3, j=4, bx=32)
                gv = g12[:].rearrange("p (j ty bx tx) -> p ty j tx bx",
                                      j=4, ty=3, bx=32, tx=3)
                for ty in range(3):
                    for j in range(4):
                        nc.vector.scalar_tensor_tensor(
                            out=gv[:, ty, j], in0=t1v[:, ty, j], scalar=mu1[:],
                            in1=t2v[:, ty, j], op0=ALU.subtract, op1=ALU.mult,
                        )
                nc.gpsimd.dma_start(out_d[0:C, sl], g12[:], accum_op=ALU.add)

            # prefetch x2 for the tail while PE runs attn@v (im pool is
            # free after the conv phase; exactly 4 slots)
            T3CHUNKS = [(0, 1152, "dve"), (1152, 3456, "act"),
                        (3456, 5760, "dve"), (5760, 8064, "act"),
                        (8064, 9216, "dve")]
            rx2_tiles = []
            for lo, hi, eng in T3CHUNKS[:4]:
                rx2 = im.tile([C, hi - lo], BF16, tag="im", name="rx2")
                nc.scalar.dma_start(rx2[:], x2f_d[:, lo:hi])
                rx2_tiles.append(rx2)
            rx2_tiles.append(None)

            EPI1_AT = {2: 0, 5: 1, 8: 2, 10: 3, 13: 4, 15: 5, 18: 6, 20: 7}
            for g in range(NG4):
                ptf = ps.tile([C, 384], F32, tag="convps", name="convps")
                pt = ptf[:, :]
                for t in range(9):
                    ky, kx = t // 3, t % 3
                    rhs = vpad[:, NROW4 * g + ky : NROW4 * g + ky + NROW4,
                               kx : kx + W]
                    nc.tensor.matmul(
                        pt[:], attnT[:, t, :], rhs, start=(t == 0), stop=(t == 8)
                    )
                sl = slice(g * NROW4 * W, (g + 1) * NROW4 * W)
                nc.vector.tensor_scalar(
                    out=y3b[:, sl], in0=pt[:], scalar1=rd[:], scalar2=0.0,
                    op0=ALU.mult, op1=ALU.add,
                    accum_out=st[:, 2, g : g + 1],
                )
                sq = scr.tile([C, 384], F32, tag="sqscr", name="sq")
                nc.scalar.activation(
                    out=sq[:], in_=pt[:], func=AF.Square, scale=rd[:],
                    accum_out=stq[:, 2, g : g + 1],
                )
                if g in EPI1_AT:
                    epi1_chunk(EPI1_AT[g])

            # ---------------- stats collective #2 (y3) ----------------
            stats2 = cst.tile([C, 2], F32)
            nc.vector.reduce_sum(stats2[:, 0:1], st[:, 2, :], axis=AX.X)
            nc.vector.reduce_sum(stats2[:, 1:2], stq[:, 2, :], axis=AX.X)
            cc2_in = dram.tile([C, 2], F32)
            cc2_out = dram.tile([C, 2], F32)
            nc.sync.dma_start(cc2_in[:], stats2[:])
            if collectives:
                nc.gpsimd.collective_compute(
                    "AllReduce", ALU.add, replica_groups=[list(range(8))],
                    ins=[cc2_in[:].opt()], outs=[cc2_out[:].opt()],
                )
            else:
                nc.sync.dma_start(cc2_out[:], cc2_in[:])
            stats2r = cst.tile([C, 2], F32)
            nc.sync.dma_start(stats2r[:], cc2_out[:])
            mu3, r3, mb3 = bn_coeffs(stats2r[:, 0:1], stats2r[:, 1:2], "y3")

            # ---------------- epilogue half 2: x2 + bn(y3) ----------------
            # t3 rotates ACT/DVE/Pool, adds rotate DVE/ACT? (ACT cannot add
            # two tensors) -> adds on DVE (bf16 2x) and Pool alternating.
            for (lo, hi, eng), rx2 in zip(T3CHUNKS, rx2_tiles):
                sl = slice(lo, hi)
                if rx2 is None:
                    rx2 = im.tile([C, hi - lo], BF16, tag="im", name="rx2")
                    nc.sync.dma_start(rx2[:], x2f_d[:, sl])
                t3 = reuse.tile([C, hi - lo], BF16, tag="t3", bufs=3,
                                padded_shape=[C, 2304], name="t3")
                if eng == "dve":
                    nc.vector.tensor_scalar(
                        out=t3[:], in0=y3b[:, sl], scalar1=mu3[:], scalar2=r3[:],
                        op0=ALU.subtract, op1=ALU.mult,
                    )
                else:
                    nc.scalar.activation(
                        t3[:], y3b[:, sl], AF.Identity, bias=mb3[:], scale=r3[:]
                    )
                nc.vector.tensor_tensor(t3[:], t3[:], rx2[:], ALU.add)
                nc.sync.dma_start(out_d[C : 2 * C, sl], t3[:])

    return nc


# alias for profiling scripts
build_nc = build_nc_v2

_CACHED_NC = None


def _get_nc():
    global _CACHED_NC
    if _CACHED_NC is None:
        _CACHED_NC = build_nc_v2()
    return _CACHED_NC


def _host_prep(x1, x2, w1, w2, wa1, wa2, wa3):
    import ml_dtypes

    bf = ml_dtypes.bfloat16

    x1f = np.ascontiguousarray(x1.reshape(B, C, HW)).astype(bf)
    x2f = np.ascontiguousarray(x2.reshape(B, C, HW)).astype(bf)

    def im2col_phase(x):
        # x: [B, C, H, W] fp32 -> [B, 128, 7, HW] bf16 im2col with rows
        # u = t*96 + ci (t = 3x3 conv tap, raster) packed as u = k*128 + p,
        # and free dim per 12-row fill ordered (ty, tx, j, bx) for output
        # pixel (12r + 3j + ty, 3bx + tx).
        xp = np.zeros((B, C, H + 2, W + 2), bf)
        xp[:, :, 1 : 1 + H, 1 : 1 + W] = x.astype(bf)
        out = np.zeros((B, 896, NF, 3, 3, 4, 32), bf)
        for ky in range(3):
            for kx in range(3):
                t = ky * 3 + kx
                v = xp[:, :, ky : ky + H, kx : kx + W]
                # rows: y = 12r + 3j + ty -> (r, j, ty); cols: x = 3bx+tx
                v = v.reshape(B, C, NF, 4, 3, 32, 3)
                # [B, ci, r, j, ty, bx, tx] -> [B, ci, r, ty, tx, j, bx]
                out[:, t * C : (t + 1) * C] = v.transpose(0, 1, 2, 4, 6, 3, 5)
        out = out.reshape(B, 7, 128, HW).transpose(0, 2, 1, 3)
        return np.ascontiguousarray(out)

    im1 = im2col_phase(x1)
    im2 = im2col_phase(x2)

    def w_im_half(w):
        # [Cout, 96, 3, 3] -> [128, 7, Cout] with row u = t*96+ci (padded)
        co, ci = w.shape[0], w.shape[1]
        u = np.transpose(w.reshape(co, ci, 9), (2, 1, 0)).reshape(9 * ci, co)
        up = np.zeros((896, co), np.float32)
        up[: 9 * ci] = u
        return np.ascontiguousarray(
            up.reshape(7, 128, co).transpose(1, 0, 2)
        ).astype(bf)

    def wqk_half(h):
        sl = slice(0, C) if h == 0 else slice(C, 2 * C)
        return np.concatenate([w_im_half(wa1[:, sl]), w_im_half(wa2[:, sl]),
                               w_im_half(wa3[:, sl])], axis=-1)

    weights = {
        "w1i": w_im_half(w1),
        "w2i": w_im_half(w2),
        "wqk": np.stack([wqk_half(0), wqk_half(1)], 1),
    }
    in_maps = []
    for b in range(B):
        m = {"x1f": x1f[b], "x2f": x2f[b], "im1": im1[b], "im2": im2[b]}
        m.update(weights)
        in_maps.append(m)
    return in_maps


def kernel(x1, x2, w1, w2, wa1, wa2, wa3):
    from concourse.bass_utils import run_bass_kernel_spmd

    x1 = np.asarray(x1, np.float32)
    x2 = np.asarray(x2, np.float32)
    in_maps = _host_prep(
        x1, x2,
        np.asarray(w1, np.float32), np.asarray(w2, np.float32),
        np.asarray(wa1, np.float32), np.asarray(wa2, np.float32),
        np.asarray(wa3, np.float32),
    )
    nc = _get_nc()
    res = run_bass_kernel_spmd(nc, in_maps, core_ids=list(range(8)))
    out0 = np.stack(
        [np.asarray(res.results[b]["out"], np.float32) for b in range(B)], 0
    ).reshape(B, 2 * C, H, W)
    out1 = np.concatenate([x1, x2], axis=1)
    return out0, out1
